# revision 9
# baseline (speedup 1.0000x reference)
"""EGNN N-body net Trainium2 Bass kernel (8-core data parallel).

Layout strategy (per core, S=512 samples):
  - node tokens feature-major: [128 feat, (i, s)] with i-major blocks of 512
  - edge tokens slot-major: 20 off-diagonal (i,j) slots; group g = i,
    jj = 0..3 indexes j != i
  - coords slot-partitioned: x_t [(i,c) 10 rows, 512]
  - e1 = W1a^T hn_i + W1b^T hn_j + W1C2^T diffsq_g accumulated in PSUM
  - attention gate via tanh identity: m*sigmoid(a) = 0.5*(1+tanh(a/2))*m,
    halves folded into x1 / h1b weights host-side; the tanh argument is
    produced partition-replicated by a replicated-lhsT matmul
  - LayerNorm via PE: centering matmul (C*g) + ones/(D g^2) variance matmul,
    ACT sqrt + DVE reciprocal; gamma/beta folded host-side
  - coordinate update via selector matmuls (F/H/G), 1/(N-1) folded into x2
  - x2 / dec2 matmuls run in bf16 (only dtypes allowing tile_position
    column offsets); everything else float32r
"""
import os
import sys
from contextlib import ExitStack

import numpy as np

for _p in ("/opt/trn_rl_repo", "/root/.axon_site/_ro/trn_rl_repo"):
    if os.path.isdir(_p) and _p not in sys.path:
        sys.path.insert(0, _p)

import ml_dtypes  # noqa: E402
import concourse.bass as bass  # noqa: E402
import concourse.bacc as bacc  # noqa: E402
import concourse.tile as tile  # noqa: E402
from concourse import mybir  # noqa: E402
from concourse.alu_op_type import AluOpType  # noqa: E402

f32 = mybir.dt.float32
f32r = mybir.dt.float32r
bf16 = mybir.dt.bfloat16
AF = mybir.ActivationFunctionType
AX = mybir.AxisListType

NPART = 5
NL = 4
BFULL = 4096
NCORES = 8
S = BFULL // NCORES          # 512 samples per core
SB = 512
D = 128
CD = 2
EPS = 1e-5

# bf16 for the edge pipeline (m1/m2/gate/x1 path). False = float32r.
EDGE_BF16 = False
EDT = bf16 if EDGE_BF16 else f32r


def _jof(g, jj):
    return jj if jj < g else jj + 1


# ---------------------------------------------------------------------------
# Param blob layout.  blob_r: f32r (node/LN/coord weights + selectors),
# blob_e: EDT (edge-MLP weights), blob_b: bf16 (col-offset matmul weights).
# ---------------------------------------------------------------------------
class _Layout:
    def __init__(self):
        self.reg = {"r": {}, "e": {}, "b": {}, "f": {}}
        self.w = {"r": 0, "e": 0, "b": 0, "f": 0}

    def add(self, blob, name, rows, cols):
        self.reg[blob][name] = (rows, self.w[blob], cols)
        self.w[blob] += cols


def _make_layout():
    L = _Layout()
    for l in range(NL):
        for nm in ("W1a", "W1b", "W2", "Wx1"):
            L.add("e", f"{nm}_{l}", 128, 128)
        for jj in range(4):
            L.add("e", f"W1C2_{l}_{jj}", 8, 128)
        if EDGE_BF16:
            L.add("e", f"watt32_{l}", 128, 32)
        else:
            L.add("e", f"wattrep_{l}", 128, 128)
        for nm in ("Wh1a", "Wh1b", "Wh2", "Wf1a", "Wf1b", "Wf2a", "Wf2b",
                   "LnC", "LnV"):
            L.add("r", f"{nm}_{l}", 128, 128)
        L.add("b", f"wx232_{l}", 128, 32)
    if EDGE_BF16:
        L.add("e", "ones_e", 128, 128)
    L.add("r", "LnC_enc", 128, 128)
    L.add("r", "LnV_enc", 128, 128)
    for g in range(NPART):
        L.add("r", f"DSel_{g}", 10, 8)
        L.add("r", f"Fg_{g}", 128, 50)
        L.add("r", f"Hg_{g}", 128, 10)
    L.add("r", "XjSel", 10, 50)
    L.add("r", "G", 128, 10)
    L.add("r", "XInit", 20, 10)
    for i in range(NPART):
        L.add("r", f"WencI_{i}", 20, 128)
    L.add("r", "Wd1", 128, 128)
    L.add("b", "Wd2", 128, 4)

    L.add("f", "identity", 128, 128)
    L.add("f", "enc_c", 128, NPART)
    L.add("f", "b_enc", 128, 1)
    for l in range(NL):
        for nm in ("b1", "b2", "bx1", "bh1", "bh2", "bf1a", "bf1b", "bf2",
                   "attb", "x2b"):
            L.add("f", f"{nm}_{l}", 128, 1)
    L.add("f", "bd1", 128, 1)
    L.add("f", "eps", 128, 1)
    L.add("f", "bd2A", 128, 1)
    L.add("f", "bd2B", 128, 1)
    return L


_L = _make_layout()


def _np(x):
    return np.asarray(x, dtype=np.float32)


def _pack(params):
    blobs = {k: np.zeros((128, _L.w[k]), np.float32) for k in "rebf"}

    def put(blob, name, arr):
        rows, c0, cols = _L.reg[blob][name]
        assert arr.shape == (rows, cols), (name, arr.shape, (rows, cols))
        blobs[blob][:rows, c0:c0 + cols] = arr

    C = np.eye(D, dtype=np.float32) - 1.0 / D

    emb = _np(params["embed"])
    ew = _np(params["enc_lin"]["w"])
    eb = _np(params["enc_lin"]["b"])
    put("f", "enc_c", ew[4:].T @ emb.T + eb[:, None])
    for i in range(NPART):
        w = np.zeros((20, 128), np.float32)
        w[4 * i:4 * i + 4, :] = ew[:4]
        put("r", f"WencI_{i}", w)
    g_enc = _np(params["enc_ln"]["g"])
    put("r", "LnC_enc", C * g_enc[None, :])
    put("r", "LnV_enc", np.repeat((1.0 / (D * g_enc ** 2))[:, None], 128, 1))
    put("f", "b_enc", _np(params["enc_ln"]["b"])[:, None])

    for l, lp in enumerate(params["layers"]):
        g = _np(lp["norm"]["g"])
        b_ln = _np(lp["norm"]["b"])
        put("r", f"LnC_{l}", C * g[None, :])
        put("r", f"LnV_{l}", np.repeat((1.0 / (D * g ** 2))[:, None], 128, 1))
        e1w = _np(lp["e1"]["w"])
        W1a, W1b, w1c = e1w[:128], e1w[128:256], e1w[256]
        put("e", f"W1a_{l}", W1a)
        put("e", f"W1b_{l}", W1b)
        for jj in range(4):
            w2c = np.zeros((8, 128), np.float32)
            w2c[2 * jj] = w1c
            w2c[2 * jj + 1] = w1c
            put("e", f"W1C2_{l}_{jj}", w2c)
        put("f", f"b1_{l}", (_np(lp["e1"]["b"]) + (W1a + W1b).T @ b_ln)[:, None])
        put("e", f"W2_{l}", _np(lp["e2"]["w"]))
        put("f", f"b2_{l}", _np(lp["e2"]["b"])[:, None])
        if EDGE_BF16:
            put("e", f"watt32_{l}", np.repeat(_np(lp["att"]["w"]), 32, 1))
        else:
            put("e", f"wattrep_{l}", np.repeat(_np(lp["att"]["w"]), 128, 1))
        put("f", f"attb_{l}",
            np.full((128, 1), 0.5 * float(_np(lp["att"]["b"])[0]), np.float32))
        put("e", f"Wx1_{l}", 0.5 * _np(lp["x1"]["w"]))
        put("f", f"bx1_{l}", _np(lp["x1"]["b"])[:, None])
        put("b", f"wx232_{l}", np.repeat(0.25 * _np(lp["x2"]["w"]), 32, 1))
        put("f", f"x2b_{l}",
            np.full((128, 1), 0.25 * float(_np(lp["x2"]["b"])[0]), np.float32))
        h1w = _np(lp["h1"]["w"])
        put("r", f"Wh1a_{l}", h1w[:128])
        put("r", f"Wh1b_{l}", 0.5 * h1w[128:])
        put("f", f"bh1_{l}", (_np(lp["h1"]["b"]) + h1w[:128].T @ b_ln)[:, None])
        put("r", f"Wh2_{l}", _np(lp["h2"]["w"]))
        put("f", f"bh2_{l}", (_np(lp["h2"]["b"]) + b_ln)[:, None])
        f1w = _np(lp["f1"]["w"])
        put("r", f"Wf1a_{l}", f1w[:, :128])
        put("r", f"Wf1b_{l}", f1w[:, 128:])
        f1b = _np(lp["f1"]["b"])
        put("f", f"bf1a_{l}", f1b[:128][:, None])
        put("f", f"bf1b_{l}", f1b[128:][:, None])
        f2w = _np(lp["f2"]["w"])
        put("r", f"Wf2a_{l}", f2w[:128])
        put("r", f"Wf2b_{l}", f2w[128:])
        put("f", f"bf2_{l}", _np(lp["f2"]["b"])[:, None])

    if EDGE_BF16:
        put("e", "ones_e", np.ones((128, 128), np.float32))
    put("f", "identity", np.eye(128, dtype=np.float32))
    put("f", "eps", np.full((128, 1), EPS, np.float32))

    for g in range(NPART):
        DSg = np.zeros((10, 8), np.float32)
        Fg = np.zeros((128, 50), np.float32)
        Hg = np.zeros((128, 10), np.float32)
        for jj in range(4):
            j = _jof(g, jj)
            for c in range(CD):
                DSg[g * 2 + c, jj * 2 + c] += 1.0
                DSg[j * 2 + c, jj * 2 + c] -= 1.0
                Fg[32 * jj, g * 10 + j * 2 + c] = 1.0
                Hg[32 * jj, g * 2 + c] = 1.0
        put("r", f"DSel_{g}", DSg)
        put("r", f"Fg_{g}", Fg)
        put("r", f"Hg_{g}", Hg)
    XjSel = np.zeros((10, 50), np.float32)
    G = np.zeros((128, 10), np.float32)
    for i in range(NPART):
        for j in range(NPART):
            for c in range(CD):
                XjSel[j * 2 + c, i * 10 + j * 2 + c] = 1.0
                G[i * 10 + j * 2 + c, i * 2 + c] = 1.0
    put("r", "XjSel", XjSel)
    put("r", "G", G)
    XI = np.zeros((20, 10), np.float32)
    for i in range(NPART):
        for c in range(CD):
            XI[i * 4 + c, i * 2 + c] = 1.0
    put("r", "XInit", XI)

    put("r", "Wd1", _np(params["dec1"]["w"]))
    put("f", "bd1", _np(params["dec1"]["b"])[:, None])
    put("b", "Wd2", _np(params["dec2"]["w"]))
    d2b = _np(params["dec2"]["b"])
    bA = np.zeros((128, 1), np.float32)
    for i in range(4):
        for c in range(4):
            bA[32 * i + c, 0] = d2b[c]
    bB = np.zeros((128, 1), np.float32)
    bB[:4, 0] = d2b
    put("f", "bd2A", bA)
    put("f", "bd2B", bB)

    blob_b = blobs["b"].astype(ml_dtypes.bfloat16)
    blob_e = (blobs["e"].astype(ml_dtypes.bfloat16) if EDGE_BF16
              else blobs["e"])
    return blobs["r"], blob_e, blob_b, blobs["f"]


# ---------------------------------------------------------------------------
# Program emission
# ---------------------------------------------------------------------------
def _emit(nc):
    state_d = nc.dram_tensor("state", [S, NPART, 4], f32,
                             kind="ExternalInput").ap()
    blobr_d = nc.dram_tensor("blob_r", [128, _L.w["r"]], f32r,
                             kind="ExternalInput").ap()
    blobe_d = nc.dram_tensor("blob_e", [128, _L.w["e"]], EDT,
                             kind="ExternalInput").ap()
    blobb_d = nc.dram_tensor("blob_b", [128, _L.w["b"]], bf16,
                             kind="ExternalInput").ap()
    blobf_d = nc.dram_tensor("blob_f", [128, _L.w["f"]], f32,
                             kind="ExternalInput").ap()
    out_d = nc.dram_tensor("out", [S, NPART, 4], f32,
                           kind="ExternalOutput").ap()

    with tile.TileContext(nc) as tc, ExitStack() as ctx:
        const = ctx.enter_context(tc.tile_pool(name="const", bufs=1))
        sbp = ctx.enter_context(tc.tile_pool(name="sbp", bufs=1))
        ring = ctx.enter_context(tc.tile_pool(name="ring", bufs=1))
        pp = ctx.enter_context(tc.tile_pool(name="pp", bufs=1, space="PSUM"))

        blob = {}
        for key, dram, dt_ in (("r", blobr_d, f32r), ("e", blobe_d, EDT),
                               ("b", blobb_d, bf16), ("f", blobf_d, f32)):
            t = const.tile([128, _L.w[key]], dt_, name=f"blob_{key}",
                           tag=f"blob_{key}")
            nc.sync.dma_start(t[:], dram)
            blob[key] = t

        def P(key, name):
            rows, c0, cols = _L.reg[key][name]
            return blob[key][0:rows, c0:c0 + cols]

        ident = P("f", "identity")

        state_sm = sbp.tile([128, 4, 20], f32)
        nc.sync.dma_start(
            state_sm[:],
            state_d.rearrange("(ch s) i c -> s ch (i c)", ch=4))

        _PB = {"pp": 5, "cwg": 1, "acc": 2}

        def psum(shape, tag="pp"):
            return pp.tile(shape, f32, tag=tag, name=tag, bufs=_PB[tag])

        # ---- state transpose -> state_t [(i,c4) 20, 512]
        state_t = sbp.tile([20, 512], f32r)
        for ch in range(4):
            ptr = psum([20, 128])
            nc.tensor.transpose(ptr[:], state_sm[:, ch, :], ident)
            nc.scalar.copy(state_t[:, ch * 128:(ch + 1) * 128], ptr[:])

        # ---- x_t init [(i,c) 10 rows, 512]
        x_pool = ctx.enter_context(tc.tile_pool(name="xp", bufs=2))
        x_t = x_pool.tile([10, 512], f32r, tag="x_t")
        px0 = psum([10, 512])
        nc.tensor.matmul(px0[:], P("r", "XInit"), state_t[:],
                         start=True, stop=True)
        nc.scalar.copy(x_t[:], px0[:])

        # ---- encoder
        h_t = sbp.tile([128, NPART * SB], f32r)
        hn_t = sbp.tile([128, NPART * SB], EDT)
        z_t = sbp.tile([128, NPART * SB], f32r, tag="z_enc")
        for i in range(NPART):
            pe = psum([128, SB])
            nc.tensor.matmul(pe[:], P("r", f"WencI_{i}"), state_t[:],
                             start=True, stop=True)
            nc.scalar.activation(z_t[:, i * SB:(i + 1) * SB], pe[:], AF.Silu,
                                 bias=P("f", "enc_c")[:, i:i + 1], scale=1.0)

        def layer_norm(src_t, dst_t, cname, vsuf, bias_ap=None):
            for i in range(NPART):
                blk = slice(i * SB, (i + 1) * SB)
                pa = psum([128, SB])
                nc.tensor.matmul(pa[:], P("r", cname), src_t[:, blk],
                                 start=True, stop=True)
                sq = ring.tile([128, SB], f32r, tag="ln_sq")
                nc.scalar.activation(sq[:], pa[:], AF.Square)
                pv = psum([128, SB])
                nc.tensor.matmul(pv[:], P("r", "LnV" + vsuf), sq[:],
                                 start=True, stop=True)
                sig = ring.tile([128, SB], f32, tag="ln_sig")
                nc.scalar.activation(sig[:], pv[:], AF.Sqrt,
                                     bias=P("f", "eps"))
                rstd = ring.tile([128, SB], f32, tag="ln_rstd")
                nc.vector.reciprocal(rstd[:], sig[:])
                if bias_ap is None:
                    nc.vector.tensor_tensor(dst_t[:, blk], pa[:], rstd[:],
                                            AluOpType.mult)
                else:
                    tmp = ring.tile([128, SB], f32, tag="ln_tmp")
                    nc.vector.tensor_tensor(tmp[:], pa[:], rstd[:],
                                            AluOpType.mult)
                    nc.scalar.activation(dst_t[:, blk], tmp[:], AF.Identity,
                                         bias=bias_ap, scale=1.0)

        layer_norm(z_t, h_t, "LnC_enc", "_enc", bias_ap=P("f", "b_enc"))

        for l in range(NL):
            layer_norm(h_t, hn_t, f"LnC_{l}", f"_{l}")

            # -- per-group squared coordinate differences
            diffsqs = []
            for g in range(NPART):
                pdiff = psum([8, 512])
                nc.tensor.matmul(pdiff[:], P("r", f"DSel_{g}"), x_t[:],
                                 start=True, stop=True)
                dsq = ring.tile([8, 512], EDT, tag=f"dsq_{g}")
                nc.scalar.activation(dsq[:], pdiff[:], AF.Square)
                diffsqs.append(dsq)
            pxj = psum([50, 512])
            nc.tensor.matmul(pxj[:], P("r", "XjSel"), x_t[:],
                             start=True, stop=True)
            xrep = ring.tile([50, 512], f32, tag="xrep")
            nc.vector.tensor_copy(xrep[:], pxj[:])

            sb_cw = []
            for g in range(NPART):
                m2s = []
                for jj in range(4):
                    j = _jof(g, jj)
                    pe1 = psum([128, 512])
                    nc.tensor.matmul(pe1[:], P("e", f"W1a_{l}"),
                                     hn_t[:, g * SB:(g + 1) * SB],
                                     start=True, stop=False)
                    nc.tensor.matmul(pe1[:], P("e", f"W1b_{l}"),
                                     hn_t[:, j * SB:(j + 1) * SB],
                                     start=False, stop=False)
                    nc.tensor.matmul(pe1[:], P("e", f"W1C2_{l}_{jj}"),
                                     diffsqs[g][:], start=False, stop=True)
                    m1 = ring.tile([128, 512], EDT, tag="m1")
                    nc.scalar.activation(m1[:], pe1[:], AF.Silu,
                                         bias=P("f", f"b1_{l}"), scale=1.0)
                    pe2 = psum([128, 512])
                    nc.tensor.matmul(pe2[:], P("e", f"W2_{l}"), m1[:],
                                     start=True, stop=True)
                    m2 = ring.tile([128, 512], EDT, tag=f"m2_{jj}")
                    nc.scalar.activation(m2[:], pe2[:], AF.Silu,
                                         bias=P("f", f"b2_{l}"), scale=1.0)
                    m2s.append(m2)
                mg = ring.tile([128, 4, 512], EDT, tag="mg")
                pcw = psum([128, 512], tag="cwg")
                for jj in range(4):
                    patt = psum([128, 512])
                    nc.tensor.matmul(patt[:], P("e", f"wattrep_{l}"),
                                     m2s[jj][:], start=True, stop=True)
                    tg = ring.tile([128, 512], EDT, tag="tg")
                    nc.scalar.activation(tg[:], patt[:], AF.Tanh,
                                         bias=P("f", f"attb_{l}"), scale=0.5)
                    nc.vector.scalar_tensor_tensor(
                        mg[:, jj, :], tg[:].bitcast(f32), 1.0,
                        m2s[jj][:].bitcast(f32),
                        AluOpType.add, AluOpType.mult)
                    px1 = psum([128, 512])
                    nc.tensor.matmul(px1[:], P("e", f"Wx1_{l}"), mg[:, jj, :],
                                     start=True, stop=True)
                    s1 = ring.tile([128, 512], bf16, tag="s1")
                    nc.scalar.activation(s1[:], px1[:], AF.Silu,
                                         bias=P("f", f"bx1_{l}"), scale=1.0)
                    nc.tensor.matmul(pcw[32 * jj:32 * jj + 32, :],
                                     P("b", f"wx232_{l}"), s1[:],
                                     start=True, stop=True,
                                     tile_position=(0, 32 * jj),
                                     skip_group_check=True)
                cwt = ring.tile([128, 512], f32r, tag=f"sbcw_{g}")
                nc.scalar.activation(cwt[:], pcw[:], AF.Identity,
                                     bias=P("f", f"x2b_{l}"), scale=1.0)
                sb_cw.append(cwt)

                # node block i = g
                agg = ring.tile([128, 512], EDT, tag="agg")
                with nc.allow_low_precision("fp32 internal accum"):
                    nc.vector.tensor_reduce(
                        agg[:],
                        mg[:].rearrange("p j s -> p s j").bitcast(f32),
                        AX.X, AluOpType.add)
                blk = slice(g * SB, (g + 1) * SB)
                ph1 = psum([128, 512])
                nc.tensor.matmul(ph1[:], P("r", f"Wh1a_{l}"),
                                 hn_t[:, blk].bitcast(f32r),
                                 start=True, stop=False)
                nc.tensor.matmul(ph1[:], P("r", f"Wh1b_{l}"),
                                 agg[:].bitcast(f32r),
                                 start=False, stop=True)
                sh1 = ring.tile([128, 512], f32r, tag="sh1")
                nc.scalar.activation(sh1[:], ph1[:], AF.Silu,
                                     bias=P("f", f"bh1_{l}"), scale=1.0)
                ph2 = psum([128, 512])
                nc.tensor.matmul(ph2[:], P("r", f"Wh2_{l}"), sh1[:],
                                 start=True, stop=True)
                tmp = ring.tile([128, 512], f32, tag="htmp")
                nc.vector.scalar_tensor_tensor(
                    tmp[:], ph2[:], P("f", f"bh2_{l}"),
                    hn_t[:, blk].bitcast(f32),
                    AluOpType.add, AluOpType.add)
                nc.vector.tensor_tensor(h_t[:, blk], h_t[:, blk].bitcast(f32),
                                        tmp[:], AluOpType.add)
                pfa = psum([128, 512])
                nc.tensor.matmul(pfa[:], P("r", f"Wf1a_{l}"), h_t[:, blk],
                                 start=True, stop=True)
                sfa = ring.tile([128, 512], f32r, tag="sfa")
                nc.scalar.activation(sfa[:], pfa[:], AF.Silu,
                                     bias=P("f", f"bf1a_{l}"), scale=1.0)
                pfb = psum([128, 512])
                nc.tensor.matmul(pfb[:], P("r", f"Wf1b_{l}"), h_t[:, blk],
                                 start=True, stop=True)
                sfb = ring.tile([128, 512], f32r, tag="sfb")
                nc.scalar.activation(sfb[:], pfb[:], AF.Silu,
                                     bias=P("f", f"bf1b_{l}"), scale=1.0)
                pf2 = psum([128, 512])
                nc.tensor.matmul(pf2[:], P("r", f"Wf2a_{l}"), sfa[:],
                                 start=True, stop=False)
                nc.tensor.matmul(pf2[:], P("r", f"Wf2b_{l}"), sfb[:],
                                 start=False, stop=True)
                nc.vector.scalar_tensor_tensor(
                    h_t[:, blk], pf2[:], P("f", f"bf2_{l}"),
                    h_t[:, blk].bitcast(f32),
                    AluOpType.add, AluOpType.add)

            # -- coordinate update
            pcwrep = psum([50, 512], tag="acc")
            ps1 = psum([10, 512], tag="acc")
            for g in range(NPART):
                nc.tensor.matmul(pcwrep[:], P("r", f"Fg_{g}"), sb_cw[g][:],
                                 start=(g == 0), stop=(g == NPART - 1))
                nc.tensor.matmul(ps1[:], P("r", f"Hg_{g}"), sb_cw[g][:],
                                 start=(g == 0), stop=(g == NPART - 1))
            prod = ring.tile([128, 512], f32r, tag="prod")
            nc.vector.memset(prod[:].bitcast(f32), 0.0)
            nc.vector.tensor_tensor(prod[0:50, :], pcwrep[:], xrep[:],
                                    AluOpType.mult)
            pt2 = psum([10, 512])
            nc.tensor.matmul(pt2[:], P("r", "G"), prod[:],
                             start=True, stop=True)
            m1x = ring.tile([10, 512], f32, tag="m1x")
            nc.vector.tensor_tensor(m1x[:], x_t[:].bitcast(f32), ps1[:],
                                    AluOpType.mult)
            dx = ring.tile([10, 512], f32, tag="dx")
            nc.vector.tensor_tensor(dx[:], m1x[:], pt2[:], AluOpType.subtract)
            x_new = x_pool.tile([10, 512], f32r, tag="x_t")
            nc.vector.tensor_tensor(x_new[:], x_t[:].bitcast(f32), dx[:],
                                    AluOpType.add)
            x_t = x_new

        # ---- decoder
        sb_decA = sbp.tile([128, 512], f32, tag="sb_decA")
        sb_decB = sbp.tile([128, 512], f32, tag="sb_decB")
        pdecA = psum([128, 512], tag="acc")
        pdecB = psum([128, 512], tag="acc")
        for i in range(NPART):
            blk = slice(i * SB, (i + 1) * SB)
            pd1 = psum([128, 512])
            nc.tensor.matmul(pd1[:], P("r", "Wd1"), h_t[:, blk],
                             start=True, stop=True)
            sd1 = ring.tile([128, 512], bf16, tag="sd1")
            nc.scalar.activation(sd1[:], pd1[:], AF.Silu,
                                 bias=P("f", "bd1"), scale=1.0)
            if i < 4:
                nc.tensor.matmul(pdecA[32 * i:32 * i + 4, :], P("b", "Wd2"),
                                 sd1[:], start=True, stop=True,
                                 tile_position=(0, 32 * i),
                                 skip_group_check=True)
            else:
                nc.tensor.matmul(pdecB[0:4, :], P("b", "Wd2"), sd1[:],
                                 start=True, stop=True)
        nc.scalar.activation(sb_decA[:], pdecA[:], AF.Identity,
                             bias=P("f", "bd2A"), scale=1.0)
        nc.scalar.activation(sb_decB[:], pdecB[:], AF.Identity,
                             bias=P("f", "bd2B"), scale=1.0)

        out_sm = sbp.tile([128, 4, 20], f32)
        for ch in range(4):
            ptA = psum([128, 128])
            nc.tensor.transpose(ptA[:], sb_decA[:, ch * 128:(ch + 1) * 128],
                                ident)
            ptB = psum([128, 128])
            nc.tensor.transpose(ptB[:], sb_decB[:, ch * 128:(ch + 1) * 128],
                                ident)
            nc.vector.tensor_tensor(
                out_sm[:, ch, 0:16].rearrange("p (i c) -> p i c", i=4),
                ptA[:].rearrange("p (i c) -> p i c", i=4)[:, :, 0:4],
                state_sm[:, ch, 0:16].rearrange("p (i c) -> p i c", i=4),
                AluOpType.add)
            nc.vector.tensor_tensor(
                out_sm[:, ch, 16:20],
                ptB[:, 0:4],
                state_sm[:, ch, 16:20],
                AluOpType.add)
        nc.sync.dma_start(
            out_d.rearrange("(ch s) i c -> s ch (i c)", ch=4), out_sm[:])


# ---------------------------------------------------------------------------
# Cached compile + PJRT runner
# ---------------------------------------------------------------------------
_RT = {}


class _Runner:
    def __init__(self):
        import jax
        from concourse import bass2jax

        nc = bacc.Bacc("TRN2", target_bir_lowering=False, debug=False,
                       num_devices=NCORES)
        _emit(nc)
        nc.compile()
        self.nc = nc

        bass2jax.install_neuronx_cc_hook()
        partition_name = (nc.partition_id_tensor.name
                          if nc.partition_id_tensor else None)
        in_names, out_names, out_avals, zero_outs = [], [], [], []
        for alloc in nc.m.functions[0].allocations:
            if not isinstance(alloc, mybir.MemoryLocationSet):
                continue
            name = alloc.memorylocations[0].name
            if alloc.kind == "ExternalInput":
                if name != partition_name:
                    in_names.append(name)
            elif alloc.kind == "ExternalOutput":
                shape = tuple(alloc.tensor_shape)
                dtype = mybir.dt.np(alloc.dtype)
                out_names.append(name)
                out_avals.append(jax.core.ShapedArray(shape, dtype))
                zero_outs.append(np.zeros(shape, dtype))
        self.in_names = list(in_names)
        self.out_names = out_names
        n_params = len(in_names)
        n_outs = len(out_avals)
        all_in_names = in_names + out_names
        if partition_name is not None:
            all_in_names.append(partition_name)
        donate = tuple(range(n_params, n_params + n_outs))

        def _body(*args):
            operands = list(args)
            if partition_name is not None:
                operands.append(bass2jax.partition_id_tensor())
            outs = bass2jax._bass_exec_p.bind(
                *operands,
                out_avals=tuple(out_avals),
                in_names=tuple(all_in_names),
                out_names=tuple(out_names),
                lowering_input_output_aliases=(),
                sim_require_finite=True,
                sim_require_nnan=True,
                nc=nc,
            )
            return tuple(outs)

        from jax.sharding import Mesh, PartitionSpec
        from jax.experimental.shard_map import shard_map

        devices = jax.devices()[:NCORES]
        mesh = Mesh(np.asarray(devices), ("core",))
        in_specs = (PartitionSpec("core"),) * (n_params + n_outs)
        out_specs = (PartitionSpec("core"),) * n_outs
        self._fn = jax.jit(
            shard_map(_body, mesh=mesh, in_specs=in_specs,
                      out_specs=out_specs, check_rep=False),
            donate_argnums=donate, keep_unused=True)
        self._zero_outs = zero_outs

    def run(self, per_core_inputs):
        concat_in = [
            np.concatenate([m[name] for m in per_core_inputs], axis=0)
            for name in self.in_names
        ]
        concat_zeros = [
            np.zeros((NCORES * z.shape[0], *z.shape[1:]), z.dtype)
            for z in self._zero_outs
        ]
        out_arrs = self._fn(*concat_in, *concat_zeros)
        return {name: np.asarray(out_arrs[i])
                for i, name in enumerate(self.out_names)}


def _get_runner():
    if "rt" not in _RT:
        _RT["rt"] = _Runner()
    return _RT["rt"]


def kernel(state, params):
    state = np.ascontiguousarray(np.asarray(state, dtype=np.float32))
    assert state.shape == (BFULL, NPART, 4), state.shape
    br, be, bb, bf_ = _pack(params)
    rt = _get_runner()
    per_core = []
    for c in range(NCORES):
        per_core.append({
            "state": state[c * S:(c + 1) * S],
            "blob_r": br,
            "blob_e": be,
            "blob_b": bb,
            "blob_f": bf_,
        })
    outs = rt.run(per_core)
    return outs["out"].astype(np.float32)
